# revision 20
# baseline (speedup 1.0000x reference)
"""Deformable multi-dilation head kernel for TRN2, 8-core row-sharded SPMD.

v2: minimal wire traffic. Per core uploads only its 16-row x shard
([2048,256] f16), a 1/8 shard of the packed weights, and a [128,8] f32
param tensor. On device: AllGather rebuilds the full [HW,C] x and the
full weights; a transpose-mode dma_gather with iota-built indices
(clamped to a zero pad row) builds the per-core halo tile for the
convs; iota generates all deformable index bases. Output returns f16.

Per core: 16 output rows (2048 pixels). Phases:
  A) 5 dilated 3x3 convs (256->27ch) via shifted matmuls on halo-padded x.
  B) per branch: index/weight math in two pixel layouts, dma_gather of 4
     bilinear corners per tap from x in [HW, C] fp16 layout, fused
     scalar_tensor_tensor MACs into a per-pixel accumulator.
  C) 1x1 conv (1536->256) over [x, br1..br5], BN stats + AllReduce, normalize.
"""
import numpy as np

try:
    # Cache XLA executables on disk: run_bass_kernel_spmd re-jits a fresh
    # closure every call, so without this every call pays a full XLA
    # recompile of the NEFF-embedding custom call (~0.5s).
    import jax
    jax.config.update("jax_compilation_cache_dir", "/tmp/jaxcache")
    jax.config.update("jax_persistent_cache_min_compile_time_secs", 0.0)
except Exception:
    pass

import concourse.bass as bass
import concourse.tile as tile
from concourse import mybir, bacc
from concourse.masks import make_identity

F32 = mybir.dt.float32
F16 = mybir.dt.float16
I32 = mybir.dt.int32
I16 = mybir.dt.int16
I8 = mybir.dt.int8
AX = mybir.AxisListType
OP = mybir.AluOpType
AF = mybir.ActivationFunctionType

H = W = 128
C = 256
DILS = (1, 6, 12, 24, 36)
NB = 5
NK = 9
RPC = H // 8          # rows per core = 16
NPIX = RPC * W        # 2048
NT = NPIX // 128      # 16
HALO = 36
HR = RPC + 2 * HALO   # 88
WP = W + 2 * HALO     # 200
NCORES = 8
NHPIX = HR * W        # halo pixels gathered per core = 11264
PADROW = H * W        # index of the zero pad row in xfull
WCOLS = NB * NK * 2 * 27 + 12 * C  # 2430 + 3072 = 5502


def build(n_cores=NCORES, acc_fp16=True):
    nc = bacc.Bacc("TRN2", num_devices=n_cores, debug=False)
    xrow = nc.dram_tensor("xrow", [NPIX, C], F16, kind="ExternalInput").ap()
    wcat = nc.dram_tensor("wcat", [16, WCOLS], F16, kind="ExternalInput").ap()
    bnp = nc.dram_tensor("bnp", [128, 8], F32, kind="ExternalInput").ap()
    out = nc.dram_tensor("out", [2, 128, RPC, W], I8, kind="ExternalOutput").ap()
    outs = nc.dram_tensor("outs", [128, 2], F32, kind="ExternalOutput").ap()

    ACC_DT = F16 if acc_fp16 else F32

    with tile.TileContext(nc) as tc:
        with tc.tile_pool(name="persist", bufs=1) as pp, \
             tc.tile_pool(name="dram", bufs=1, space="DRAM") as dram:

            ident = pp.tile([128, 128], F32)
            make_identity(nc, ident[:])
            ident16 = pp.tile([128, 128], F16)
            nc.vector.tensor_copy(out=ident16[:], in_=ident[:])

            bn_sb = pp.tile([128, 8], F32)
            nc.sync.dma_start(out=bn_sb[:], in_=bnp[:])

            dfoT1 = pp.tile([128, NB, NT, 27], F32)
            dfoT2 = pp.tile([128, NB, 16, 18], F32)
            accT = pp.tile([128, NB, 2, NPIX], F16)
            xT = pp.tile([128, 2, NPIX], F16)

            # ---------- Phase 0: AllGather x + weights; build halo tile ----
            xrow_d = dram.tile([NPIX, C], F16)
            nc.sync.dma_start(out=xrow_d[:], in_=xrow[:])
            xfull = dram.tile([H * W + 1, C], F16)
            nc.gpsimd.collective_compute(
                "AllGather", OP.bypass,
                replica_groups=[list(range(n_cores))],
                ins=[xrow_d[:]], outs=[xfull[0:H * W, :]],
            )
            zrow = pp.tile([128, C], F16)
            nc.vector.memset(zrow[:], 0.0)
            nc.sync.dma_start(out=xfull[PADROW:PADROW + 1, :], in_=zrow[0:1, :])

            wcat_d = dram.tile([16, WCOLS], F16)
            nc.sync.dma_start(out=wcat_d[:], in_=wcat[:])
            wfull = dram.tile([128, WCOLS], F16)
            nc.gpsimd.collective_compute(
                "AllGather", OP.bypass,
                replica_groups=[list(range(n_cores))],
                ins=[wcat_d[:]], outs=[wfull[:]],
            )

            # ---------------- Phase A: convolutions ----------------
            with tc.tile_pool(name="convp", bufs=1) as cp, \
                 tc.tile_pool(name="psA1", bufs=1, space="PSUM") as psA1, \
                 tc.tile_pool(name="psA2", bufs=2, space="PSUM") as psA2:
                # halo indices: idxflat[i] = 128*(h0-36) + i, i in [0, NHPIX),
                # out-of-image rows -> PADROW. wrapped layout [p, c]:
                # value at (p, c) for i = 16c + p%16 -> iota(p + 16c) + bnp[:,7]
                # where bnp[:,7] = 128*(h0-36) - 16*(p//16).
                hi_f = cp.tile([128, NHPIX // 16], F32)
                nc.gpsimd.iota(hi_f[:], pattern=[[16, NHPIX // 16]], base=0,
                               channel_multiplier=1,
                               allow_small_or_imprecise_dtypes=True)
                nc.vector.tensor_scalar(out=hi_f[:], in0=hi_f[:],
                                        scalar1=bn_sb[:, 7:8], scalar2=None, op0=OP.add)
                hi_m = cp.tile([128, NHPIX // 16], F32)
                nc.vector.tensor_scalar(out=hi_m[:], in0=hi_f[:],
                                        scalar1=0.0, scalar2=None, op0=OP.is_ge)
                nc.vector.tensor_scalar(out=hi_f[:], in0=hi_f[:],
                                        scalar1=float(PADROW), scalar2=None, op0=OP.subtract)
                nc.vector.tensor_tensor(out=hi_f[:], in0=hi_f[:], in1=hi_m[:], op=OP.mult)
                nc.vector.tensor_scalar(out=hi_f[:], in0=hi_f[:],
                                        scalar1=float(PADROW), scalar2=float(PADROW),
                                        op0=OP.add, op1=OP.min)
                hi_idx = cp.tile([128, NHPIX // 16], I16)
                nc.vector.tensor_copy(out=hi_idx[:], in_=hi_f[:])

                GCH = 512           # 4 image rows per chunk; s2m=66 < 128 FIFO
                NCH = NHPIX // GCH  # 22
                halo_t = cp.tile([128, NCH, 2, GCH], F16)
                for ci in range(NCH):
                    nc.gpsimd.dma_gather(
                        halo_t[:, ci], xfull[:],
                        hi_idx[:, ci * (GCH // 16):(ci + 1) * (GCH // 16)],
                        GCH, GCH, C, transpose=True,
                    )

                xs = cp.tile([128, 2, HR, WP], F16)
                nc.vector.memset(xs[:], 0.0)
                for cc in range(2):
                    nc.vector.tensor_copy(
                        out=xs[:, cc, :, HALO:HALO + W].rearrange(
                            "p (ci h) w -> p ci h w", h=4),
                        in_=halo_t[:, :, cc].rearrange("p ci (h w) -> p ci h w", w=W))
                    nc.vector.tensor_copy(
                        out=xT[:, cc].rearrange("p (h w) -> p h w", w=W),
                        in_=xs[:, cc, HALO:HALO + RPC, HALO:HALO + W])

                wcs = cp.tile([128, NB * NK * 2, 27], F16)
                nc.sync.dma_start(
                    out=wcs[:],
                    in_=wfull[:, 0:NB * NK * 2 * 27].rearrange("p (a b) -> p a b", b=27))

                for b in range(NB):
                    d = DILS[b]
                    psum_dfo = psA1.tile([27, NPIX], F32, tag="psdfo")
                    for r4 in range(RPC // 4):
                        for k in range(NK):
                            ky, kx = k // 3, k % 3
                            dy, dx = (ky - 1) * d, (kx - 1) * d
                            for cc in range(2):
                                # rhs: 4 rows per matmul (512 psum cols = 1 bank)
                                nc.tensor.matmul(
                                    psum_dfo[:, r4 * 512:(r4 + 1) * 512],
                                    lhsT=wcs[:, (b * NK + k) * 2 + cc, :],
                                    rhs=xs[:, cc, HALO + dy + 4 * r4:HALO + dy + 4 * r4 + 4,
                                           HALO + dx:HALO + dx + W],
                                    start=(k == 0 and cc == 0),
                                    stop=(k == NK - 1 and cc == 1),
                                )
                    dfo_sb = cp.tile([27, NPIX], F32, tag="dfosb")
                    nc.scalar.copy(out=dfo_sb[:], in_=psum_dfo[:])
                    # layout-1 transposes: [27, 128] chunks -> [128, 27], x4 batched
                    for t4 in range(NT // 4):
                        pt = psA2.tile([128, 4, 27], F32, tag="pst1")
                        for j in range(4):
                            nc.tensor.transpose(
                                pt[:, j], dfo_sb[:, (4 * t4 + j) * 128:(4 * t4 + j + 1) * 128],
                                ident[:27, :27])
                        nc.scalar.copy(out=dfoT1[:, b, 4 * t4:4 * t4 + 4, :], in_=pt[:])
                    # layout-2 transposes: strided chunks (pixels q, q+16, ...)
                    dview = dfo_sb[:].rearrange("c (s q) -> c q s", q=16)
                    for q4 in range(4):
                        pt2 = psA2.tile([128, 4, 18], F32, tag="pst2")
                        for j in range(4):
                            nc.tensor.transpose(pt2[:, j], dview[:, 4 * q4 + j, :],
                                                ident[:27, :18])
                        nc.scalar.copy(out=dfoT2[:, b, 4 * q4:4 * q4 + 4, :], in_=pt2[:])

            # ---------------- Phase B: gather + MAC per branch ----------------
            with tc.tile_pool(name="mathp", bufs=2) as mp, \
                 tc.tile_pool(name="gathp", bufs=3) as gp, \
                 tc.tile_pool(name="accp", bufs=1) as ap_, \
                 tc.tile_pool(name="psB", bufs=2, space="PSUM") as psB:

                for b in range(NB):
                    d = DILS[b]
                    # ---- index bases via iota (replaces host giota) ----
                    g1y = mp.tile([128, NK, NT], F32, tag="g1y")
                    nc.gpsimd.iota(g1y[:], pattern=[[d, 3], [0, 3], [1, 16]],
                                   base=-d, channel_multiplier=0,
                                   allow_small_or_imprecise_dtypes=True)
                    nc.vector.tensor_scalar(out=g1y[:], in0=g1y[:],
                                            scalar1=bn_sb[:, 4:5], scalar2=None, op0=OP.add)
                    g1x = mp.tile([128, NK, NT], F32, tag="g1x")
                    nc.gpsimd.iota(g1x[:], pattern=[[0, 3], [d, 3], [0, 16]],
                                   base=-d, channel_multiplier=1,
                                   allow_small_or_imprecise_dtypes=True)
                    g2y = mp.tile([128, NK, 16], F32, tag="g2y")
                    nc.gpsimd.iota(g2y[:], pattern=[[d, 3], [0, 3], [0, 16]],
                                   base=-d, channel_multiplier=0,
                                   allow_small_or_imprecise_dtypes=True)
                    nc.vector.tensor_scalar(out=g2y[:], in0=g2y[:],
                                            scalar1=bn_sb[:, 5:6], scalar2=None, op0=OP.add)
                    g2x = mp.tile([128, NK, 16], F32, tag="g2x")
                    nc.gpsimd.iota(g2x[:], pattern=[[0, 3], [d, 3], [1, 16]],
                                   base=-d, channel_multiplier=0,
                                   allow_small_or_imprecise_dtypes=True)
                    nc.vector.tensor_scalar(out=g2x[:], in0=g2x[:],
                                            scalar1=bn_sb[:, 6:7], scalar2=None, op0=OP.add)

                    # ---- layout-1 math (weights) ----
                    d1 = dfoT1[:, b].rearrange("p t c -> p c t")
                    py = mp.tile([128, NK, NT], F32, tag="py")
                    px = mp.tile([128, NK, NT], F32, tag="px")
                    nc.vector.tensor_tensor(out=py[:], in0=d1[:, 0:9, :], in1=g1y[:], op=OP.add)
                    nc.vector.tensor_tensor(out=px[:], in0=d1[:, 9:18, :], in1=g1x[:], op=OP.add)
                    ee = mp.tile([128, NK, NT], F32, tag="ee")
                    nc.scalar.activation(out=ee[:], in_=d1[:, 18:27, :], func=AF.Exp)
                    # sumexp over taps (tree) then reciprocal
                    se = mp.tile([128, 4, NT], F32, tag="se")
                    nc.vector.tensor_tensor(out=se[:, 0], in0=ee[:, 0], in1=ee[:, 1], op=OP.add)
                    nc.vector.tensor_tensor(out=se[:, 1], in0=ee[:, 2], in1=ee[:, 3], op=OP.add)
                    nc.vector.tensor_tensor(out=se[:, 2], in0=ee[:, 4], in1=ee[:, 5], op=OP.add)
                    nc.vector.tensor_tensor(out=se[:, 3], in0=ee[:, 6], in1=ee[:, 7], op=OP.add)
                    nc.vector.tensor_tensor(out=se[:, 0], in0=se[:, 0], in1=se[:, 1], op=OP.add)
                    nc.vector.tensor_tensor(out=se[:, 2], in0=se[:, 2], in1=se[:, 3], op=OP.add)
                    nc.vector.tensor_tensor(out=se[:, 0], in0=se[:, 0], in1=se[:, 2], op=OP.add)
                    nc.vector.tensor_tensor(out=se[:, 0], in0=se[:, 0], in1=ee[:, 8], op=OP.add)
                    rec = mp.tile([128, NT], F32, tag="rec")
                    nc.vector.reciprocal(out=rec[:], in_=se[:, 0])

                    def frac_weights(pos, tagpfx):
                        """returns (w_lo, w_hi) each [128, NK, NT] incl. validity."""
                        i0 = mp.tile([128, NK, NT], I32, tag=tagpfx + "i0")
                        nc.vector.tensor_scalar(out=i0[:], in0=pos[:], scalar1=0.5, scalar2=None, op0=OP.subtract)
                        f0 = mp.tile([128, NK, NT], F32, tag=tagpfx + "f0")
                        nc.vector.tensor_copy(out=f0[:], in_=i0[:])
                        whi = mp.tile([128, NK, NT], F32, tag=tagpfx + "whi")
                        nc.vector.tensor_tensor(out=whi[:], in0=pos[:], in1=f0[:], op=OP.subtract)
                        wlo = mp.tile([128, NK, NT], F32, tag=tagpfx + "wlo")
                        nc.vector.tensor_scalar(out=wlo[:], in0=whi[:], scalar1=1.0, scalar2=-1.0, op0=OP.subtract, op1=OP.mult)
                        c0 = mp.tile([128, NK, NT], F32, tag=tagpfx + "c0")
                        nc.vector.tensor_scalar(out=c0[:], in0=f0[:], scalar1=0.0, scalar2=127.0, op0=OP.max, op1=OP.min)
                        v0 = mp.tile([128, NK, NT], F32, tag=tagpfx + "v0")
                        nc.vector.tensor_tensor(out=v0[:], in0=c0[:], in1=f0[:], op=OP.is_equal)
                        f1 = mp.tile([128, NK, NT], F32, tag=tagpfx + "f1")
                        nc.vector.tensor_scalar(out=f1[:], in0=f0[:], scalar1=1.0, scalar2=None, op0=OP.add)
                        c1 = mp.tile([128, NK, NT], F32, tag=tagpfx + "c1")
                        nc.vector.tensor_scalar(out=c1[:], in0=f1[:], scalar1=0.0, scalar2=127.0, op0=OP.max, op1=OP.min)
                        v1 = mp.tile([128, NK, NT], F32, tag=tagpfx + "v1")
                        nc.vector.tensor_tensor(out=v1[:], in0=c1[:], in1=f1[:], op=OP.is_equal)
                        a0 = mp.tile([128, NK, NT], F32, tag=tagpfx + "a0")
                        nc.vector.tensor_tensor(out=a0[:], in0=wlo[:], in1=v0[:], op=OP.mult)
                        a1 = mp.tile([128, NK, NT], F32, tag=tagpfx + "a1")
                        nc.vector.tensor_tensor(out=a1[:], in0=whi[:], in1=v1[:], op=OP.mult)
                        return a0, a1

                    ay0, ay1 = frac_weights(py, "y")
                    bx0, bx1 = frac_weights(px, "x")
                    # fold exp * recip into y weights
                    er = mp.tile([128, NK, NT], F32, tag="er")
                    rb = rec[:].unsqueeze(1).broadcast_to([128, NK, NT])
                    nc.vector.tensor_tensor(out=er[:], in0=ee[:], in1=rb, op=OP.mult)
                    nc.vector.tensor_tensor(out=ay0[:], in0=ay0[:], in1=er[:], op=OP.mult)
                    nc.vector.tensor_tensor(out=ay1[:], in0=ay1[:], in1=er[:], op=OP.mult)
                    wts = []
                    for (wy, tg) in ((ay0, "w0"), (ay1, "w1")):
                        for (wx, tg2) in ((bx0, "a"), (bx1, "b")):
                            wt = mp.tile([128, NK, NT], F32, tag=tg + tg2)
                            nc.vector.tensor_tensor(out=wt[:], in0=wy[:], in1=wx[:], op=OP.mult)
                            wts.append(wt)

                    # ---- layout-2 math (indices) ----
                    d2 = dfoT2[:, b].rearrange("p q c -> p c q")
                    py2 = mp.tile([128, NK, 16], F32, tag="py2")
                    px2 = mp.tile([128, NK, 16], F32, tag="px2")
                    nc.vector.tensor_tensor(out=py2[:], in0=d2[:, 0:9, :], in1=g2y[:], op=OP.add)
                    nc.vector.tensor_tensor(out=px2[:], in0=d2[:, 9:18, :], in1=g2x[:], op=OP.add)

                    def corner_idx(pos, tagpfx):
                        """returns (c0f, c1f): clipped floor / floor+1 coords as f32."""
                        i0 = mp.tile([128, NK, 16], I32, tag=tagpfx + "2i0")
                        nc.vector.tensor_scalar(out=i0[:], in0=pos[:], scalar1=0.5, scalar2=None, op0=OP.subtract)
                        f0 = mp.tile([128, NK, 16], F32, tag=tagpfx + "2f0")
                        nc.vector.tensor_copy(out=f0[:], in_=i0[:])
                        c0 = mp.tile([128, NK, 16], F32, tag=tagpfx + "2c0")
                        nc.vector.tensor_scalar(out=c0[:], in0=f0[:], scalar1=0.0, scalar2=127.0, op0=OP.max, op1=OP.min)
                        c1 = mp.tile([128, NK, 16], F32, tag=tagpfx + "2c1")
                        nc.vector.tensor_scalar(out=c1[:], in0=f0[:], scalar1=1.0, scalar2=127.0, op0=OP.add, op1=OP.min)
                        nc.vector.tensor_scalar(out=c1[:], in0=c1[:], scalar1=0.0, scalar2=None, op0=OP.max)
                        return c0, c1

                    yc0, yc1 = corner_idx(py2, "y")
                    xc0, xc1 = corner_idx(px2, "x")
                    qidx = []
                    for (yc, tg) in ((yc0, "q0"), (yc1, "q1")):
                        for (xc, tg2) in ((xc0, "a"), (xc1, "b")):
                            qi = mp.tile([128, NK, 16], F32, tag=tg + tg2)
                            nc.vector.scalar_tensor_tensor(out=qi[:], in0=yc[:], scalar=float(W), in1=xc[:], op0=OP.mult, op1=OP.add)
                            qidx.append(qi)

                    # ---- gathers + MACs ----
                    acc = ap_.tile([128, NT, C], ACC_DT, tag="acc")
                    firstmac = True

                    def build_idx_half(qi, k, idx16, half):
                        rep = gp.tile([128, 8, 16], F32, tag="rep")
                        nc.gpsimd.tensor_copy(
                            out=rep[:],
                            in_=qi[:, k, :].unsqueeze(1).broadcast_to([128, 8, 16]),
                        )
                        tp = psB.tile([128, 128], F32, tag="idxt")
                        nc.tensor.transpose(tp[:], rep[:].rearrange("p a b -> p (a b)"), ident[:])
                        nc.vector.tensor_copy(
                            out=idx16[:, half * 128:(half + 1) * 128], in_=tp[:])

                    def macs(gdst, wt, k, toff):
                        # one tap at a time: wb broadcasts wt[p, k, t] along C
                        nonlocal firstmac
                        gslice = gdst[:, toff:toff + NT, :]
                        wb = wt[:, k, :].unsqueeze(2).broadcast_to([128, NT, C])
                        if firstmac:
                            nc.vector.tensor_tensor(out=acc[:], in0=gslice, in1=wb, op=OP.mult)
                        else:
                            nc.vector.tensor_tensor(out=gslice, in0=gslice, in1=wb, op=OP.mult)
                            nc.vector.tensor_tensor(out=acc[:], in0=acc[:], in1=gslice, op=OP.add)
                        firstmac = False

                    for ci in range(4):
                        wt = wts[ci]
                        qi = qidx[ci]
                        # 4 tap-pairs batched (2 taps per dma_gather), tap 8 single
                        for ka in range(0, 8, 2):
                            idx16 = gp.tile([128, 256], I16, tag="idx16p")
                            build_idx_half(qi, ka, idx16, 0)
                            build_idx_half(qi, ka + 1, idx16, 1)
                            gdst = gp.tile([128, 2 * NT, C], F16, tag="gdstp")
                            nc.gpsimd.dma_gather(
                                gdst[:], xfull[:], idx16[:], 2 * NPIX, 2 * NPIX, C,
                                single_packet=False,
                            )
                            macs(gdst, wt, ka, 0)
                            macs(gdst, wt, ka + 1, NT)
                        idx16s = gp.tile([128, 128], I16, tag="idx16s")
                        build_idx_half(qi, 8, idx16s, 0)
                        gdsts = gp.tile([128, NT, C], F16, tag="gdsts")
                        nc.gpsimd.dma_gather(
                            gdsts[:], xfull[:], idx16s[:], NPIX, NPIX, C,
                            single_packet=False,
                        )
                        macs(gdsts, wt, 8, 0)

                    # ---- transpose acc -> [ch, pix] fp16, x4 batched copies ----
                    for cc in range(2):
                        for t4 in range(NT // 4):
                            tp2 = psB.tile([128, 4, 128], ACC_DT, tag="accTt")
                            for j in range(4):
                                nc.tensor.transpose(
                                    tp2[:, j], acc[:, 4 * t4 + j, cc * 128:(cc + 1) * 128],
                                    ident16[:] if ACC_DT == F16 else ident[:])
                            nc.scalar.copy(
                                out=accT[:, b, cc, 4 * t4 * 128:(4 * t4 + 4) * 128],
                                in_=tp2[:].rearrange("p a b -> p (a b)"))

            # ---------------- Phase C: 1x1 conv + BN ----------------
            with tc.tile_pool(name="finp", bufs=1) as fp, \
                 tc.tile_pool(name="psC", bufs=1, space="PSUM") as psC:
                ws_sb = fp.tile([128, 12, C], F16)
                nc.sync.dma_start(
                    out=ws_sb[:],
                    in_=wfull[:, NB * NK * 2 * 27:].rearrange("p (a b) -> p a b", b=C))

                rhs_chunks = [xT[:, 0, :], xT[:, 1, :]]
                for b in range(NB):
                    rhs_chunks += [accT[:, b, 0, :], accT[:, b, 1, :]]

                y_sb = fp.tile([128, 2, NPIX], F32)
                stats4 = fp.tile([128, 4], F32)
                scratch = fp.tile([128, NPIX], F32)
                for cc in range(2):
                    psum_y = psC.tile([128, NPIX], F32, tag="psy")
                    for pb in range(4):
                        for ci in range(12):
                            nc.tensor.matmul(
                                psum_y[:, pb * 512:(pb + 1) * 512],
                                lhsT=ws_sb[:, ci, cc * 128:(cc + 1) * 128],
                                rhs=rhs_chunks[ci][:, pb * 512:(pb + 1) * 512],
                                start=(ci == 0), stop=(ci == 11),
                            )
                    nc.vector.tensor_copy(out=y_sb[:, cc, :], in_=psum_y[:])
                    nc.vector.tensor_reduce(out=stats4[:, 2 * cc:2 * cc + 1], in_=y_sb[:, cc, :], axis=AX.X, op=OP.add)
                    nc.scalar.activation(out=scratch[:], in_=y_sb[:, cc, :], func=AF.Square,
                                         accum_out=stats4[:, 2 * cc + 1:2 * cc + 2])

                db_in = dram.tile([128, 4], F32)
                db_out = dram.tile([128, 4], F32)
                nc.sync.dma_start(out=db_in[:], in_=stats4[:])
                nc.gpsimd.collective_compute(
                    "AllReduce", OP.add,
                    replica_groups=[list(range(n_cores))],
                    ins=[db_in[:]], outs=[db_out[:]],
                )
                statsr = fp.tile([128, 4], F32)
                nc.sync.dma_start(out=statsr[:], in_=db_out[:])

                NPIXTOT = float(H * W)
                sview = statsr[:].rearrange("p (a b) -> p b a", b=2)
                mean = fp.tile([128, 2], F32)
                nc.vector.tensor_scalar(out=mean[:], in0=sview[:, 0, :], scalar1=1.0 / NPIXTOT, scalar2=None, op0=OP.mult)
                var = fp.tile([128, 2], F32)
                nc.vector.tensor_scalar(out=var[:], in0=sview[:, 1, :], scalar1=1.0 / NPIXTOT, scalar2=None, op0=OP.mult)
                msq = fp.tile([128, 2], F32)
                nc.vector.tensor_tensor(out=msq[:], in0=mean[:], in1=mean[:], op=OP.mult)
                nc.vector.tensor_tensor(out=var[:], in0=var[:], in1=msq[:], op=OP.subtract)
                epst = fp.tile([128, 1], F32)
                nc.vector.memset(epst[:], 1e-5)
                rs = fp.tile([128, 2], F32)
                nc.scalar.activation(out=rs[:], in_=var[:], func=AF.Sqrt, bias=epst[:])
                nc.vector.reciprocal(out=rs[:], in_=rs[:])
                aa = fp.tile([128, 2], F32)
                nc.vector.tensor_tensor(out=aa[:], in0=rs[:], in1=bn_sb[:, 0:2], op=OP.mult)
                bb = fp.tile([128, 2], F32)
                nc.vector.tensor_tensor(out=bb[:], in0=mean[:], in1=aa[:], op=OP.mult)
                nc.vector.tensor_tensor(out=bb[:], in0=bn_sb[:, 2:4], in1=bb[:], op=OP.subtract)
                # int8 output with per-(channel, cc) scale: bounded quant
                # error <= amax/254 (~0.4% of channel max).
                amax = fp.tile([128, 2], F32)
                amin = fp.tile([128, 2], F32)
                rsc = fp.tile([128, 2], F32)
                outq = fp.tile([128, 2, NPIX], I8)
                for cc in range(2):
                    nc.vector.tensor_scalar(
                        out=y_sb[:, cc, :], in0=y_sb[:, cc, :],
                        scalar1=aa[:, cc:cc + 1], scalar2=bb[:, cc:cc + 1],
                        op0=OP.mult, op1=OP.add)
                    nc.vector.tensor_reduce(
                        out=amax[:, cc:cc + 1], in_=y_sb[:, cc, :],
                        axis=AX.X, op=OP.max)
                    nc.vector.tensor_reduce(
                        out=amin[:, cc:cc + 1], in_=y_sb[:, cc, :],
                        axis=AX.X, op=OP.min)
                nc.vector.tensor_scalar(out=amin[:], in0=amin[:],
                                        scalar1=-1.0, scalar2=None, op0=OP.mult)
                nc.vector.tensor_tensor(out=amax[:], in0=amax[:], in1=amin[:], op=OP.max)
                nc.vector.reciprocal(out=rsc[:], in_=amax[:])
                nc.vector.tensor_scalar(out=rsc[:], in0=rsc[:],
                                        scalar1=127.0, scalar2=None, op0=OP.mult)
                for cc in range(2):
                    nc.vector.tensor_scalar(
                        out=y_sb[:, cc, :], in0=y_sb[:, cc, :],
                        scalar1=rsc[:, cc:cc + 1], scalar2=None, op0=OP.mult)
                    nc.vector.tensor_copy(out=outq[:, cc, :], in_=y_sb[:, cc, :])
                    nc.sync.dma_start(
                        out=out[cc],
                        in_=outq[:, cc, :].rearrange("p (h w) -> p h w", w=W))
                scl = fp.tile([128, 2], F32)
                nc.vector.tensor_scalar(out=scl[:], in0=amax[:],
                                        scalar1=1.0 / 127.0, scalar2=None, op0=OP.mult)
                nc.sync.dma_start(out=outs[:], in_=scl[:])
    nc.compile()
    return nc


def prep_inputs(x, ws, w_scale, bn_weight, bn_bias):
    """Host-side: build per-core input maps. x: [1,C,H,W] f32; ws: list of 5 [27,C,3,3]."""
    x = np.asarray(x)[0]  # [C, H, W]
    x16 = x.astype(np.float16)
    x_hwc = np.ascontiguousarray(x16.reshape(C, H * W).T)  # [HW, C]

    # conv weights: out-channel perm [dy(9), dx(9), f(9)]; final [128, NB*NK*2, 27]
    perm = [9 + 2 * k for k in range(9)] + [10 + 2 * k for k in range(9)] + list(range(9))
    wconv = np.zeros((128, NB * NK * 2, 27), np.float16)
    for b in range(NB):
        wb = np.asarray(ws[b])[perm]  # [27, C, 3, 3]
        for k in range(NK):
            ky, kx = k // 3, k % 3
            m = wb[:, :, ky, kx]  # [27, C]
            wconv[:, (b * NK + k) * 2 + 0, :] = m[:, :128].T.astype(np.float16)
            wconv[:, (b * NK + k) * 2 + 1, :] = m[:, 128:].T.astype(np.float16)

    wsT = np.ascontiguousarray(
        np.asarray(w_scale)[:, :, 0, 0].T.astype(np.float16).reshape(12, 128, C)
        .transpose(1, 0, 2))  # [128, 12, 256]

    wcat_full = np.concatenate(
        [wconv.reshape(128, -1), wsT.reshape(128, -1)], axis=1)  # [128, 5502] f16

    P = np.arange(128)
    in_maps = []
    for core in range(NCORES):
        h0 = core * RPC
        bnp = np.zeros((128, 8), np.float32)
        bnp[:, 0] = bn_weight[:128]
        bnp[:, 1] = bn_weight[128:]
        bnp[:, 2] = bn_bias[:128]
        bnp[:, 3] = bn_bias[128:]
        bnp[:, 4] = h0
        bnp[:, 5] = h0 + P // 8
        bnp[:, 6] = 16 * (P % 8)
        bnp[:, 7] = 128.0 * (h0 - HALO) - 16 * (P // 16)

        in_maps.append(dict(
            xrow=np.ascontiguousarray(x_hwc[core * NPIX:(core + 1) * NPIX]),
            wcat=np.ascontiguousarray(wcat_full[core * 16:(core + 1) * 16]),
            bnp=bnp,
        ))
    return in_maps


def assemble_output(results):
    """results: 8 dicts with 'out' [2,128,RPC,W] i8 + 'outs' [128,2] f32 scales."""
    y = np.zeros((1, C, H, W), np.float32)
    for core, r in enumerate(results):
        o = np.asarray(r["out"], dtype=np.float32)
        s = np.asarray(r["outs"], dtype=np.float32)  # [128, 2]
        y[0, :128, core * RPC:(core + 1) * RPC, :] = o[0] * s[:, 0][:, None, None]
        y[0, 128:, core * RPC:(core + 1) * RPC, :] = o[1] * s[:, 1][:, None, None]
    return y


# ----------------------------------------------------------------------------
# Public entry point: kernel(**inputs) -> np.ndarray
# ----------------------------------------------------------------------------
_NC_CACHE = {}


def _get_nc():
    if "nc" not in _NC_CACHE:
        _NC_CACHE["nc"] = build()
    return _NC_CACHE["nc"]


def kernel(x, w1, w2, w3, w4, w5, w_scale, bn_weight, bn_bias):
    from concourse.bass_utils import run_bass_kernel_spmd
    nc = _get_nc()
    in_maps = prep_inputs(
        np.asarray(x, dtype=np.float32),
        [np.asarray(w, dtype=np.float32) for w in (w1, w2, w3, w4, w5)],
        np.asarray(w_scale, dtype=np.float32),
        np.asarray(bn_weight, dtype=np.float32),
        np.asarray(bn_bias, dtype=np.float32),
    )
    res = run_bass_kernel_spmd(nc, in_maps, core_ids=list(range(NCORES)))
    return assemble_output(res.results)


# revision 24
# speedup vs baseline: 1.1418x; 1.1418x over previous
"""Deformable multi-dilation head kernel for TRN2, 8-core row-sharded SPMD.

v2: minimal wire traffic. Per core uploads only its 16-row x shard
([2048,256] f16), a 1/8 shard of the packed weights, and a [128,8] f32
param tensor. On device: AllGather rebuilds the full [HW,C] x and the
full weights; a transpose-mode dma_gather with iota-built indices
(clamped to a zero pad row) builds the per-core halo tile for the
convs; iota generates all deformable index bases. Output returns f16.

Per core: 16 output rows (2048 pixels). Phases:
  A) 5 dilated 3x3 convs (256->27ch) via shifted matmuls on halo-padded x.
  B) per branch: index/weight math in two pixel layouts, dma_gather of 4
     bilinear corners per tap from x in [HW, C] fp16 layout, fused
     scalar_tensor_tensor MACs into a per-pixel accumulator.
  C) 1x1 conv (1536->256) over [x, br1..br5], BN stats + AllReduce, normalize.
"""
import numpy as np

try:
    # Cache XLA executables on disk: run_bass_kernel_spmd re-jits a fresh
    # closure every call, so without this every call pays a full XLA
    # recompile of the NEFF-embedding custom call (~0.5s).
    import jax
    jax.config.update("jax_compilation_cache_dir", "/tmp/jaxcache")
    jax.config.update("jax_persistent_cache_min_compile_time_secs", 0.0)
except Exception:
    pass

import concourse.bass as bass
import concourse.tile as tile
from concourse import mybir, bacc
from concourse.masks import make_identity

F32 = mybir.dt.float32
F16 = mybir.dt.float16
I32 = mybir.dt.int32
I16 = mybir.dt.int16
I8 = mybir.dt.int8
AX = mybir.AxisListType
OP = mybir.AluOpType
AF = mybir.ActivationFunctionType

H = W = 128
C = 256
DILS = (1, 6, 12, 24, 36)
NB = 5
NK = 9
RPC = H // 8          # rows per core = 16
NPIX = RPC * W        # 2048
NT = NPIX // 128      # 16
HALO = 36
HR = RPC + 2 * HALO   # 88
WP = W + 2 * HALO     # 200
NCORES = 8
NHPIX = HR * W        # halo pixels gathered per core = 11264
PADROW = H * W        # index of the zero pad row in xfull
WCOLS = NB * NK * 2 * 27 + 12 * C  # 2430 + 3072 = 5502


def build(n_cores=NCORES, acc_fp16=True):
    nc = bacc.Bacc("TRN2", num_devices=n_cores, debug=False, num_swdge_queues=4)
    xrow = nc.dram_tensor("xrow", [NPIX, C], F16, kind="ExternalInput").ap()
    wcat = nc.dram_tensor("wcat", [16, WCOLS], F16, kind="ExternalInput").ap()
    bnp = nc.dram_tensor("bnp", [128, 8], F32, kind="ExternalInput").ap()
    out = nc.dram_tensor("out", [2, 128, RPC, W], I8, kind="ExternalOutput").ap()
    outs = nc.dram_tensor("outs", [128, 2], F32, kind="ExternalOutput").ap()

    ACC_DT = F16 if acc_fp16 else F32

    with tile.TileContext(nc) as tc:
        with tc.tile_pool(name="persist", bufs=1) as pp, \
             tc.tile_pool(name="dram", bufs=1, space="DRAM") as dram:

            ident = pp.tile([128, 128], F32)
            make_identity(nc, ident[:])
            ident16 = pp.tile([128, 128], F16)
            nc.vector.tensor_copy(out=ident16[:], in_=ident[:])

            bn_sb = pp.tile([128, 8], F32)
            nc.sync.dma_start(out=bn_sb[:], in_=bnp[:])

            dfoT1 = pp.tile([128, NB, NT, 27], F32)
            dfoT2 = pp.tile([128, NB, 16, 18], F32)
            accT = pp.tile([128, NB, 2, NPIX], F16)
            xT = pp.tile([128, 2, NPIX], F16)

            # ---------- Phase 0: AllGather x + weights; build halo tile ----
            xrow_d = dram.tile([NPIX, C], F16)
            nc.sync.dma_start(out=xrow_d[:], in_=xrow[:])
            xfull = dram.tile([H * W + 1, C], F16)
            nc.gpsimd.collective_compute(
                "AllGather", OP.bypass,
                replica_groups=[list(range(n_cores))],
                ins=[xrow_d[:]], outs=[xfull[0:H * W, :]],
            )
            zrow = pp.tile([128, C], F16)
            nc.vector.memset(zrow[:], 0.0)
            nc.sync.dma_start(out=xfull[PADROW:PADROW + 1, :], in_=zrow[0:1, :])

            wcat_d = dram.tile([16, WCOLS], F16)
            nc.sync.dma_start(out=wcat_d[:], in_=wcat[:])
            wfull = dram.tile([128, WCOLS], F16)
            nc.gpsimd.collective_compute(
                "AllGather", OP.bypass,
                replica_groups=[list(range(n_cores))],
                ins=[wcat_d[:]], outs=[wfull[:]],
            )

            # ---------------- Phase A: convolutions ----------------
            with tc.tile_pool(name="convp", bufs=1) as cp, \
                 tc.tile_pool(name="psA1", bufs=1, space="PSUM") as psA1, \
                 tc.tile_pool(name="psA2", bufs=2, space="PSUM") as psA2:
                # halo indices: idxflat[i] = 128*(h0-36) + i, i in [0, NHPIX),
                # out-of-image rows -> PADROW. wrapped layout [p, c]:
                # value at (p, c) for i = 16c + p%16 -> iota(p + 16c) + bnp[:,7]
                # where bnp[:,7] = 128*(h0-36) - 16*(p//16).
                hi_f = cp.tile([128, NHPIX // 16], F32)
                nc.gpsimd.iota(hi_f[:], pattern=[[16, NHPIX // 16]], base=0,
                               channel_multiplier=1,
                               allow_small_or_imprecise_dtypes=True)
                nc.vector.tensor_scalar(out=hi_f[:], in0=hi_f[:],
                                        scalar1=bn_sb[:, 7:8], scalar2=None, op0=OP.add)
                hi_m = cp.tile([128, NHPIX // 16], F32)
                nc.vector.tensor_scalar(out=hi_m[:], in0=hi_f[:],
                                        scalar1=0.0, scalar2=None, op0=OP.is_ge)
                nc.vector.tensor_scalar(out=hi_f[:], in0=hi_f[:],
                                        scalar1=float(PADROW), scalar2=None, op0=OP.subtract)
                nc.vector.tensor_tensor(out=hi_f[:], in0=hi_f[:], in1=hi_m[:], op=OP.mult)
                nc.vector.tensor_scalar(out=hi_f[:], in0=hi_f[:],
                                        scalar1=float(PADROW), scalar2=float(PADROW),
                                        op0=OP.add, op1=OP.min)
                hi_idx = cp.tile([128, NHPIX // 16], I16)
                nc.vector.tensor_copy(out=hi_idx[:], in_=hi_f[:])

                GCH = 512           # 4 image rows per chunk; s2m=66 < 128 FIFO
                NCH = NHPIX // GCH  # 22
                halo_t = cp.tile([128, NCH, 2, GCH], F16)
                for ci in range(NCH):
                    nc.gpsimd.dma_gather(
                        halo_t[:, ci], xfull[:],
                        hi_idx[:, ci * (GCH // 16):(ci + 1) * (GCH // 16)],
                        GCH, GCH, C, transpose=True, queue_num=ci % 4,
                    )

                xs = cp.tile([128, 2, HR, WP], F16)
                nc.vector.memset(xs[:], 0.0)
                for cc in range(2):
                    nc.vector.tensor_copy(
                        out=xs[:, cc, :, HALO:HALO + W].rearrange(
                            "p (ci h) w -> p ci h w", h=4),
                        in_=halo_t[:, :, cc].rearrange("p ci (h w) -> p ci h w", w=W))
                    nc.vector.tensor_copy(
                        out=xT[:, cc].rearrange("p (h w) -> p h w", w=W),
                        in_=xs[:, cc, HALO:HALO + RPC, HALO:HALO + W])

                wcs = cp.tile([128, NB * NK * 2, 27], F16)
                nc.sync.dma_start(
                    out=wcs[:],
                    in_=wfull[:, 0:NB * NK * 2 * 27].rearrange("p (a b) -> p a b", b=27))

                for b in range(NB):
                    d = DILS[b]
                    psum_dfo = psA1.tile([27, NPIX], F32, tag="psdfo")
                    for r4 in range(RPC // 4):
                        for k in range(NK):
                            ky, kx = k // 3, k % 3
                            dy, dx = (ky - 1) * d, (kx - 1) * d
                            for cc in range(2):
                                # rhs: 4 rows per matmul (512 psum cols = 1 bank)
                                nc.tensor.matmul(
                                    psum_dfo[:, r4 * 512:(r4 + 1) * 512],
                                    lhsT=wcs[:, (b * NK + k) * 2 + cc, :],
                                    rhs=xs[:, cc, HALO + dy + 4 * r4:HALO + dy + 4 * r4 + 4,
                                           HALO + dx:HALO + dx + W],
                                    start=(k == 0 and cc == 0),
                                    stop=(k == NK - 1 and cc == 1),
                                )
                    dfo_sb = cp.tile([27, NPIX], F32, tag="dfosb")
                    nc.scalar.copy(out=dfo_sb[:], in_=psum_dfo[:])
                    # layout-1 transposes: [27, 128] chunks -> [128, 27], x4 batched
                    for t4 in range(NT // 4):
                        pt = psA2.tile([128, 4, 27], F32, tag="pst1")
                        for j in range(4):
                            nc.tensor.transpose(
                                pt[:, j], dfo_sb[:, (4 * t4 + j) * 128:(4 * t4 + j + 1) * 128],
                                ident[:27, :27])
                        nc.scalar.copy(out=dfoT1[:, b, 4 * t4:4 * t4 + 4, :], in_=pt[:])
                    # layout-2 transposes: strided chunks (pixels q, q+16, ...)
                    dview = dfo_sb[:].rearrange("c (s q) -> c q s", q=16)
                    for q4 in range(4):
                        pt2 = psA2.tile([128, 4, 18], F32, tag="pst2")
                        for j in range(4):
                            nc.tensor.transpose(pt2[:, j], dview[:, 4 * q4 + j, :],
                                                ident[:27, :18])
                        nc.scalar.copy(out=dfoT2[:, b, 4 * q4:4 * q4 + 4, :], in_=pt2[:])

            # ---------------- Phase B: gather + MAC per branch ----------------
            with tc.tile_pool(name="mathp", bufs=2) as mp, \
                 tc.tile_pool(name="gathp", bufs=3) as gp, \
                 tc.tile_pool(name="accp", bufs=1) as ap_, \
                 tc.tile_pool(name="psB", bufs=2, space="PSUM") as psB:

                for b in range(NB):
                    d = DILS[b]
                    # ---- index bases via iota (replaces host giota) ----
                    g1y = mp.tile([128, NK, NT], F32, tag="g1y")
                    nc.gpsimd.iota(g1y[:], pattern=[[d, 3], [0, 3], [1, 16]],
                                   base=-d, channel_multiplier=0,
                                   allow_small_or_imprecise_dtypes=True)
                    nc.vector.tensor_scalar(out=g1y[:], in0=g1y[:],
                                            scalar1=bn_sb[:, 4:5], scalar2=None, op0=OP.add)
                    g1x = mp.tile([128, NK, NT], F32, tag="g1x")
                    nc.gpsimd.iota(g1x[:], pattern=[[0, 3], [d, 3], [0, 16]],
                                   base=-d, channel_multiplier=1,
                                   allow_small_or_imprecise_dtypes=True)
                    g2y = mp.tile([128, NK, 16], F32, tag="g2y")
                    nc.gpsimd.iota(g2y[:], pattern=[[d, 3], [0, 3], [0, 16]],
                                   base=-d, channel_multiplier=0,
                                   allow_small_or_imprecise_dtypes=True)
                    nc.vector.tensor_scalar(out=g2y[:], in0=g2y[:],
                                            scalar1=bn_sb[:, 5:6], scalar2=None, op0=OP.add)
                    g2x = mp.tile([128, NK, 16], F32, tag="g2x")
                    nc.gpsimd.iota(g2x[:], pattern=[[0, 3], [d, 3], [1, 16]],
                                   base=-d, channel_multiplier=0,
                                   allow_small_or_imprecise_dtypes=True)
                    nc.vector.tensor_scalar(out=g2x[:], in0=g2x[:],
                                            scalar1=bn_sb[:, 6:7], scalar2=None, op0=OP.add)

                    # ---- layout-1 math (weights) ----
                    d1 = dfoT1[:, b].rearrange("p t c -> p c t")
                    py = mp.tile([128, NK, NT], F32, tag="py")
                    px = mp.tile([128, NK, NT], F32, tag="px")
                    nc.vector.tensor_tensor(out=py[:], in0=d1[:, 0:9, :], in1=g1y[:], op=OP.add)
                    nc.vector.tensor_tensor(out=px[:], in0=d1[:, 9:18, :], in1=g1x[:], op=OP.add)
                    ee = mp.tile([128, NK, NT], F32, tag="ee")
                    nc.scalar.activation(out=ee[:], in_=d1[:, 18:27, :], func=AF.Exp)
                    # sumexp over taps (tree) then reciprocal
                    se = mp.tile([128, 4, NT], F32, tag="se")
                    nc.vector.tensor_tensor(out=se[:, 0], in0=ee[:, 0], in1=ee[:, 1], op=OP.add)
                    nc.vector.tensor_tensor(out=se[:, 1], in0=ee[:, 2], in1=ee[:, 3], op=OP.add)
                    nc.vector.tensor_tensor(out=se[:, 2], in0=ee[:, 4], in1=ee[:, 5], op=OP.add)
                    nc.vector.tensor_tensor(out=se[:, 3], in0=ee[:, 6], in1=ee[:, 7], op=OP.add)
                    nc.vector.tensor_tensor(out=se[:, 0], in0=se[:, 0], in1=se[:, 1], op=OP.add)
                    nc.vector.tensor_tensor(out=se[:, 2], in0=se[:, 2], in1=se[:, 3], op=OP.add)
                    nc.vector.tensor_tensor(out=se[:, 0], in0=se[:, 0], in1=se[:, 2], op=OP.add)
                    nc.vector.tensor_tensor(out=se[:, 0], in0=se[:, 0], in1=ee[:, 8], op=OP.add)
                    rec = mp.tile([128, NT], F32, tag="rec")
                    nc.vector.reciprocal(out=rec[:], in_=se[:, 0])

                    def frac_weights(pos, tagpfx):
                        """returns (w_lo, w_hi) each [128, NK, NT] incl. validity."""
                        i0 = mp.tile([128, NK, NT], I32, tag=tagpfx + "i0")
                        nc.vector.tensor_scalar(out=i0[:], in0=pos[:], scalar1=0.5, scalar2=None, op0=OP.subtract)
                        f0 = mp.tile([128, NK, NT], F32, tag=tagpfx + "f0")
                        nc.vector.tensor_copy(out=f0[:], in_=i0[:])
                        whi = mp.tile([128, NK, NT], F32, tag=tagpfx + "whi")
                        nc.vector.tensor_tensor(out=whi[:], in0=pos[:], in1=f0[:], op=OP.subtract)
                        wlo = mp.tile([128, NK, NT], F32, tag=tagpfx + "wlo")
                        nc.vector.tensor_scalar(out=wlo[:], in0=whi[:], scalar1=1.0, scalar2=-1.0, op0=OP.subtract, op1=OP.mult)
                        c0 = mp.tile([128, NK, NT], F32, tag=tagpfx + "c0")
                        nc.vector.tensor_scalar(out=c0[:], in0=f0[:], scalar1=0.0, scalar2=127.0, op0=OP.max, op1=OP.min)
                        v0 = mp.tile([128, NK, NT], F32, tag=tagpfx + "v0")
                        nc.vector.tensor_tensor(out=v0[:], in0=c0[:], in1=f0[:], op=OP.is_equal)
                        f1 = mp.tile([128, NK, NT], F32, tag=tagpfx + "f1")
                        nc.vector.tensor_scalar(out=f1[:], in0=f0[:], scalar1=1.0, scalar2=None, op0=OP.add)
                        c1 = mp.tile([128, NK, NT], F32, tag=tagpfx + "c1")
                        nc.vector.tensor_scalar(out=c1[:], in0=f1[:], scalar1=0.0, scalar2=127.0, op0=OP.max, op1=OP.min)
                        v1 = mp.tile([128, NK, NT], F32, tag=tagpfx + "v1")
                        nc.vector.tensor_tensor(out=v1[:], in0=c1[:], in1=f1[:], op=OP.is_equal)
                        a0 = mp.tile([128, NK, NT], F32, tag=tagpfx + "a0")
                        nc.vector.tensor_tensor(out=a0[:], in0=wlo[:], in1=v0[:], op=OP.mult)
                        a1 = mp.tile([128, NK, NT], F32, tag=tagpfx + "a1")
                        nc.vector.tensor_tensor(out=a1[:], in0=whi[:], in1=v1[:], op=OP.mult)
                        return a0, a1

                    ay0, ay1 = frac_weights(py, "y")
                    bx0, bx1 = frac_weights(px, "x")
                    # fold exp * recip into y weights
                    er = mp.tile([128, NK, NT], F32, tag="er")
                    rb = rec[:].unsqueeze(1).broadcast_to([128, NK, NT])
                    nc.vector.tensor_tensor(out=er[:], in0=ee[:], in1=rb, op=OP.mult)
                    nc.vector.tensor_tensor(out=ay0[:], in0=ay0[:], in1=er[:], op=OP.mult)
                    nc.vector.tensor_tensor(out=ay1[:], in0=ay1[:], in1=er[:], op=OP.mult)
                    wts = []
                    for (wy, tg) in ((ay0, "w0"), (ay1, "w1")):
                        for (wx, tg2) in ((bx0, "a"), (bx1, "b")):
                            wt = mp.tile([128, NK, NT], F32, tag=tg + tg2)
                            nc.vector.tensor_tensor(out=wt[:], in0=wy[:], in1=wx[:], op=OP.mult)
                            wts.append(wt)

                    # ---- layout-2 math (indices) ----
                    d2 = dfoT2[:, b].rearrange("p q c -> p c q")
                    py2 = mp.tile([128, NK, 16], F32, tag="py2")
                    px2 = mp.tile([128, NK, 16], F32, tag="px2")
                    nc.vector.tensor_tensor(out=py2[:], in0=d2[:, 0:9, :], in1=g2y[:], op=OP.add)
                    nc.vector.tensor_tensor(out=px2[:], in0=d2[:, 9:18, :], in1=g2x[:], op=OP.add)

                    def corner_idx(pos, tagpfx):
                        """returns (c0f, c1f): clipped floor / floor+1 coords as f32."""
                        i0 = mp.tile([128, NK, 16], I32, tag=tagpfx + "2i0")
                        nc.vector.tensor_scalar(out=i0[:], in0=pos[:], scalar1=0.5, scalar2=None, op0=OP.subtract)
                        f0 = mp.tile([128, NK, 16], F32, tag=tagpfx + "2f0")
                        nc.vector.tensor_copy(out=f0[:], in_=i0[:])
                        c0 = mp.tile([128, NK, 16], F32, tag=tagpfx + "2c0")
                        nc.vector.tensor_scalar(out=c0[:], in0=f0[:], scalar1=0.0, scalar2=127.0, op0=OP.max, op1=OP.min)
                        c1 = mp.tile([128, NK, 16], F32, tag=tagpfx + "2c1")
                        nc.vector.tensor_scalar(out=c1[:], in0=f0[:], scalar1=1.0, scalar2=127.0, op0=OP.add, op1=OP.min)
                        nc.vector.tensor_scalar(out=c1[:], in0=c1[:], scalar1=0.0, scalar2=None, op0=OP.max)
                        return c0, c1

                    yc0, yc1 = corner_idx(py2, "y")
                    xc0, xc1 = corner_idx(px2, "x")
                    qidx = []
                    for (yc, tg) in ((yc0, "q0"), (yc1, "q1")):
                        for (xc, tg2) in ((xc0, "a"), (xc1, "b")):
                            qi = mp.tile([128, NK, 16], F32, tag=tg + tg2)
                            nc.vector.scalar_tensor_tensor(out=qi[:], in0=yc[:], scalar=float(W), in1=xc[:], op0=OP.mult, op1=OP.add)
                            qidx.append(qi)

                    # ---- gathers + MACs ----
                    acc = ap_.tile([128, NT, C], ACC_DT, tag="acc")
                    firstmac = True

                    def build_idx_half(qi, k, idx16, half):
                        rep = gp.tile([128, 8, 16], F32, tag="rep")
                        nc.gpsimd.tensor_copy(
                            out=rep[:],
                            in_=qi[:, k, :].unsqueeze(1).broadcast_to([128, 8, 16]),
                        )
                        tp = psB.tile([128, 128], F32, tag="idxt")
                        nc.tensor.transpose(tp[:], rep[:].rearrange("p a b -> p (a b)"), ident[:])
                        nc.vector.tensor_copy(
                            out=idx16[:, half * 128:(half + 1) * 128], in_=tp[:])

                    def macs(gdst, wt, k, toff):
                        # one tap at a time: wb broadcasts wt[p, k, t] along C
                        nonlocal firstmac
                        gslice = gdst[:, toff:toff + NT, :]
                        wb = wt[:, k, :].unsqueeze(2).broadcast_to([128, NT, C])
                        if firstmac:
                            nc.vector.tensor_tensor(out=acc[:], in0=gslice, in1=wb, op=OP.mult)
                        else:
                            nc.vector.tensor_tensor(out=gslice, in0=gslice, in1=wb, op=OP.mult)
                            nc.vector.tensor_tensor(out=acc[:], in0=acc[:], in1=gslice, op=OP.add)
                        firstmac = False

                    gq = 0
                    for ci in range(4):
                        wt = wts[ci]
                        qi = qidx[ci]
                        # 4 tap-pairs batched (2 taps per dma_gather), tap 8 single
                        for ka in range(0, 8, 2):
                            idx16 = gp.tile([128, 256], I16, tag="idx16p")
                            build_idx_half(qi, ka, idx16, 0)
                            build_idx_half(qi, ka + 1, idx16, 1)
                            gdst = gp.tile([128, 2 * NT, C], F16, tag="gdstp")
                            nc.gpsimd.dma_gather(
                                gdst[:], xfull[:], idx16[:], 2 * NPIX, 2 * NPIX, C,
                                single_packet=False, queue_num=gq % 4,
                            )
                            gq += 1
                            macs(gdst, wt, ka, 0)
                            macs(gdst, wt, ka + 1, NT)
                        idx16s = gp.tile([128, 128], I16, tag="idx16s")
                        build_idx_half(qi, 8, idx16s, 0)
                        gdsts = gp.tile([128, NT, C], F16, tag="gdsts")
                        nc.gpsimd.dma_gather(
                            gdsts[:], xfull[:], idx16s[:], NPIX, NPIX, C,
                            single_packet=False, queue_num=gq % 4,
                        )
                        gq += 1
                        macs(gdsts, wt, 8, 0)

                    # ---- transpose acc -> [ch, pix] fp16, x4 batched copies ----
                    for cc in range(2):
                        for t4 in range(NT // 4):
                            tp2 = psB.tile([128, 4, 128], ACC_DT, tag="accTt")
                            for j in range(4):
                                nc.tensor.transpose(
                                    tp2[:, j], acc[:, 4 * t4 + j, cc * 128:(cc + 1) * 128],
                                    ident16[:] if ACC_DT == F16 else ident[:])
                            nc.scalar.copy(
                                out=accT[:, b, cc, 4 * t4 * 128:(4 * t4 + 4) * 128],
                                in_=tp2[:].rearrange("p a b -> p (a b)"))

            # ---------------- Phase C: 1x1 conv + BN ----------------
            with tc.tile_pool(name="finp", bufs=1) as fp, \
                 tc.tile_pool(name="psC", bufs=1, space="PSUM") as psC:
                ws_sb = fp.tile([128, 12, C], F16)
                nc.sync.dma_start(
                    out=ws_sb[:],
                    in_=wfull[:, NB * NK * 2 * 27:].rearrange("p (a b) -> p a b", b=C))

                rhs_chunks = [xT[:, 0, :], xT[:, 1, :]]
                for b in range(NB):
                    rhs_chunks += [accT[:, b, 0, :], accT[:, b, 1, :]]

                y_sb = fp.tile([128, 2, NPIX], F32)
                stats4 = fp.tile([128, 4], F32)
                scratch = fp.tile([128, NPIX], F32)
                for cc in range(2):
                    psum_y = psC.tile([128, NPIX], F32, tag="psy")
                    for pb in range(4):
                        for ci in range(12):
                            nc.tensor.matmul(
                                psum_y[:, pb * 512:(pb + 1) * 512],
                                lhsT=ws_sb[:, ci, cc * 128:(cc + 1) * 128],
                                rhs=rhs_chunks[ci][:, pb * 512:(pb + 1) * 512],
                                start=(ci == 0), stop=(ci == 11),
                            )
                    nc.vector.tensor_copy(out=y_sb[:, cc, :], in_=psum_y[:])
                    nc.vector.tensor_reduce(out=stats4[:, 2 * cc:2 * cc + 1], in_=y_sb[:, cc, :], axis=AX.X, op=OP.add)
                    nc.scalar.activation(out=scratch[:], in_=y_sb[:, cc, :], func=AF.Square,
                                         accum_out=stats4[:, 2 * cc + 1:2 * cc + 2])

                db_in = dram.tile([128, 4], F32)
                db_out = dram.tile([128, 4], F32)
                nc.sync.dma_start(out=db_in[:], in_=stats4[:])
                nc.gpsimd.collective_compute(
                    "AllReduce", OP.add,
                    replica_groups=[list(range(n_cores))],
                    ins=[db_in[:]], outs=[db_out[:]],
                )
                statsr = fp.tile([128, 4], F32)
                nc.sync.dma_start(out=statsr[:], in_=db_out[:])

                NPIXTOT = float(H * W)
                sview = statsr[:].rearrange("p (a b) -> p b a", b=2)
                mean = fp.tile([128, 2], F32)
                nc.vector.tensor_scalar(out=mean[:], in0=sview[:, 0, :], scalar1=1.0 / NPIXTOT, scalar2=None, op0=OP.mult)
                var = fp.tile([128, 2], F32)
                nc.vector.tensor_scalar(out=var[:], in0=sview[:, 1, :], scalar1=1.0 / NPIXTOT, scalar2=None, op0=OP.mult)
                msq = fp.tile([128, 2], F32)
                nc.vector.tensor_tensor(out=msq[:], in0=mean[:], in1=mean[:], op=OP.mult)
                nc.vector.tensor_tensor(out=var[:], in0=var[:], in1=msq[:], op=OP.subtract)
                epst = fp.tile([128, 1], F32)
                nc.vector.memset(epst[:], 1e-5)
                rs = fp.tile([128, 2], F32)
                nc.scalar.activation(out=rs[:], in_=var[:], func=AF.Sqrt, bias=epst[:])
                nc.vector.reciprocal(out=rs[:], in_=rs[:])
                aa = fp.tile([128, 2], F32)
                nc.vector.tensor_tensor(out=aa[:], in0=rs[:], in1=bn_sb[:, 0:2], op=OP.mult)
                bb = fp.tile([128, 2], F32)
                nc.vector.tensor_tensor(out=bb[:], in0=mean[:], in1=aa[:], op=OP.mult)
                nc.vector.tensor_tensor(out=bb[:], in0=bn_sb[:, 2:4], in1=bb[:], op=OP.subtract)
                # int8 output with per-(channel, cc) scale: bounded quant
                # error <= amax/254 (~0.4% of channel max).
                amax = fp.tile([128, 2], F32)
                amin = fp.tile([128, 2], F32)
                rsc = fp.tile([128, 2], F32)
                outq = fp.tile([128, 2, NPIX], I8)
                for cc in range(2):
                    nc.vector.tensor_scalar(
                        out=y_sb[:, cc, :], in0=y_sb[:, cc, :],
                        scalar1=aa[:, cc:cc + 1], scalar2=bb[:, cc:cc + 1],
                        op0=OP.mult, op1=OP.add)
                    nc.vector.tensor_reduce(
                        out=amax[:, cc:cc + 1], in_=y_sb[:, cc, :],
                        axis=AX.X, op=OP.max)
                    nc.vector.tensor_reduce(
                        out=amin[:, cc:cc + 1], in_=y_sb[:, cc, :],
                        axis=AX.X, op=OP.min)
                nc.vector.tensor_scalar(out=amin[:], in0=amin[:],
                                        scalar1=-1.0, scalar2=None, op0=OP.mult)
                nc.vector.tensor_tensor(out=amax[:], in0=amax[:], in1=amin[:], op=OP.max)
                nc.vector.reciprocal(out=rsc[:], in_=amax[:])
                nc.vector.tensor_scalar(out=rsc[:], in0=rsc[:],
                                        scalar1=127.0, scalar2=None, op0=OP.mult)
                for cc in range(2):
                    nc.vector.tensor_scalar(
                        out=y_sb[:, cc, :], in0=y_sb[:, cc, :],
                        scalar1=rsc[:, cc:cc + 1], scalar2=None, op0=OP.mult)
                    nc.vector.tensor_copy(out=outq[:, cc, :], in_=y_sb[:, cc, :])
                    nc.sync.dma_start(
                        out=out[cc],
                        in_=outq[:, cc, :].rearrange("p (h w) -> p h w", w=W))
                scl = fp.tile([128, 2], F32)
                nc.vector.tensor_scalar(out=scl[:], in0=amax[:],
                                        scalar1=1.0 / 127.0, scalar2=None, op0=OP.mult)
                nc.sync.dma_start(out=outs[:], in_=scl[:])
    nc.compile()
    return nc


def prep_inputs(x, ws, w_scale, bn_weight, bn_bias):
    """Host-side: build per-core input maps. x: [1,C,H,W] f32; ws: list of 5 [27,C,3,3]."""
    x = np.asarray(x)[0]  # [C, H, W]
    x16 = x.astype(np.float16)
    x_hwc = np.ascontiguousarray(x16.reshape(C, H * W).T)  # [HW, C]

    # conv weights: out-channel perm [dy(9), dx(9), f(9)]; final [128, NB*NK*2, 27]
    perm = [9 + 2 * k for k in range(9)] + [10 + 2 * k for k in range(9)] + list(range(9))
    w5 = np.stack([np.asarray(ws[b])[perm] for b in range(NB)])  # [5, 27, C, 3, 3]
    # wconv[p, (b*9+ky*3+kx)*2+cc, o] = w5[b, o, cc*128+p, ky, kx]
    wconv = np.ascontiguousarray(
        w5.transpose(0, 3, 4, 2, 1)               # [5, 3, 3, C, 27]
        .reshape(NB * NK, 2, 128, 27)
        .transpose(2, 0, 1, 3)
        .reshape(128, NB * NK * 2, 27).astype(np.float16))

    wsT = np.ascontiguousarray(
        np.asarray(w_scale)[:, :, 0, 0].T.astype(np.float16).reshape(12, 128, C)
        .transpose(1, 0, 2))  # [128, 12, 256]

    wcat_full = np.concatenate(
        [wconv.reshape(128, -1), wsT.reshape(128, -1)], axis=1)  # [128, 5502] f16

    P = np.arange(128)
    in_maps = []
    for core in range(NCORES):
        h0 = core * RPC
        bnp = np.zeros((128, 8), np.float32)
        bnp[:, 0] = bn_weight[:128]
        bnp[:, 1] = bn_weight[128:]
        bnp[:, 2] = bn_bias[:128]
        bnp[:, 3] = bn_bias[128:]
        bnp[:, 4] = h0
        bnp[:, 5] = h0 + P // 8
        bnp[:, 6] = 16 * (P % 8)
        bnp[:, 7] = 128.0 * (h0 - HALO) - 16 * (P // 16)

        in_maps.append(dict(
            xrow=np.ascontiguousarray(x_hwc[core * NPIX:(core + 1) * NPIX]),
            wcat=np.ascontiguousarray(wcat_full[core * 16:(core + 1) * 16]),
            bnp=bnp,
        ))
    return in_maps


def assemble_output(results):
    """results: 8 dicts with 'out' [2,128,RPC,W] i8 + 'outs' [128,2] f32 scales."""
    y = np.zeros((1, C, H, W), np.float32)
    for core, r in enumerate(results):
        o = np.asarray(r["out"], dtype=np.float32)
        s = np.asarray(r["outs"], dtype=np.float32)  # [128, 2]
        y[0, :128, core * RPC:(core + 1) * RPC, :] = o[0] * s[:, 0][:, None, None]
        y[0, 128:, core * RPC:(core + 1) * RPC, :] = o[1] * s[:, 1][:, None, None]
    return y


# ----------------------------------------------------------------------------
# Public entry point: kernel(**inputs) -> np.ndarray
# ----------------------------------------------------------------------------
_NC_CACHE = {}


def _get_nc():
    if "nc" not in _NC_CACHE:
        _NC_CACHE["nc"] = build()
    return _NC_CACHE["nc"]


def kernel(x, w1, w2, w3, w4, w5, w_scale, bn_weight, bn_bias):
    from concourse.bass_utils import run_bass_kernel_spmd
    nc = _get_nc()
    in_maps = prep_inputs(
        np.asarray(x, dtype=np.float32),
        [np.asarray(w, dtype=np.float32) for w in (w1, w2, w3, w4, w5)],
        np.asarray(w_scale, dtype=np.float32),
        np.asarray(bn_weight, dtype=np.float32),
        np.asarray(bn_bias, dtype=np.float32),
    )
    res = run_bass_kernel_spmd(nc, in_maps, core_ids=list(range(NCORES)))
    return assemble_output(res.results)
